# revision 39
# baseline (speedup 1.0000x reference)
"""XL-BOMD rank-4 Krylov propagation (EnergyXL) on 8 TRN2 NeuronCores.

Data-parallel over molecules: 512 mols -> 64 per core.  The operator
A(v) = R v R - v is self-adjoint w.r.t. the Frobenius inner product, so
the reference's full Gram-Schmidt chain collapses to an (unnormalized)
Lanczos 3-term recurrence:

  p_0 = D - P,  n_k = <p_k,p_k>
  W'  = R p_k R              (bf16 PE sandwiches, fp32 PSUM accum)
  a'_k = <W', p_k> / n_k,  b_k = n_k / n_{k-1}
  p_{k+1} = W' - a'_k p_k - b_k p_{k-1}

The final dP2dt2 = -V (W^T W)^-1 W^T dDS is basis-invariant over the
Krylov subspace; O/c are reconstructed from the tridiagonal scalars
(a_k = a'_k - 1, beta_k = sqrt(n_k/n_{k-1})); rank 3 needs no p_4:
O_33 = (<W',W'> - 2 S_3 + n_3)/n_3.  Batched symmetric 4x4 solve per
8-mol block, then out = -sum_k y_k/sqrt(n_k) p_k.

Layout: each 192x192 matrix lives as one [96, 384] SBUF tile (rows
0:96 -> cols 0:192, rows 96:192 -> cols 192:384), so every elementwise
op / reduction is a single instruction.  Matmuls split into 4 [96,96]
weight chunks x 192-wide moving ops into a single [96,384] PSUM bank.
Vectors are stored bf16 (validated 2.6e-3 rel err vs 2e-2 budget);
all reductions accumulate fp32.
"""

import sys

sys.path.insert(0, "/opt/trn_rl_repo")

import numpy as np

import concourse.bass as bass
import concourse.bacc as bacc
import concourse.tile as tile
from concourse import mybir
from concourse.bass_utils import run_bass_kernel_spmd

F32 = mybir.dt.float32
BF16 = mybir.dt.bfloat16
ALU = mybir.AluOpType
ACTF = mybir.ActivationFunctionType

NMOL, N, RANK = 512, 192, 4
NCORES = 8
MPC = NMOL // NCORES  # 64 molecules per core
HP = 96               # partitions per tile (192 rows in 2 col groups)
FW = 384              # free width: 2 x 192
BLK = 16              # molecules per solve block

# partials cols (per mol, [96, 9]): n_k at 2k (k=0..3), S_k at 2k+1, WW at 8
NPART = 9


def build_core_kernel(n_mols=MPC):
    nc = bacc.Bacc(None, target_bir_lowering=False, enable_partition_id=False)
    D = nc.dram_tensor("D", [n_mols, N, N], F32, kind="ExternalInput")
    P = nc.dram_tensor("P", [n_mols, N, N], F32, kind="ExternalInput")
    R = nc.dram_tensor("Rm", [n_mols, N, N], F32, kind="ExternalInput")
    OUT = nc.dram_tensor("OUT", [n_mols, N, N], F32, kind="ExternalOutput")

    with tile.TileContext(nc) as tc:
        _body(nc, tc, D, P, R, OUT)
    nc.finalize()
    return nc


def _load_e(nc, t, X, m):
    """DMA X[m] (192x192 DRAM) into E-layout tile t [96, 384]."""
    nc.sync.dma_start(out=t[:, 0:N], in_=X[m, 0:HP, :])
    nc.sync.dma_start(out=t[:, N:FW], in_=X[m, HP:N, :])


def _store_e(nc, X, m, t):
    nc.sync.dma_start(out=X[m, 0:HP, :], in_=t[:, 0:N])
    nc.sync.dma_start(out=X[m, HP:N, :], in_=t[:, N:FW])


def _sandwich(nc, ps, L, B):
    """ps[96,384] (PSUM) = (L @ B) in E-layout; L symmetric, both bf16.

    out rows 0:96 -> ps[:,0:192], rows 96:192 -> ps[:,192:384];
    contraction split over row groups 0:96 / 96:192.
    """
    mm = nc.tensor.matmul
    mm(ps[:, 0:N], lhsT=L[:, 0:HP], rhs=B[:, 0:N], start=True, stop=False)
    mm(ps[:, 0:N], lhsT=L[:, 2 * HP:3 * HP], rhs=B[:, N:FW], start=False, stop=True)
    mm(ps[:, N:FW], lhsT=L[:, HP:2 * HP], rhs=B[:, 0:N], start=True, stop=False)
    mm(ps[:, N:FW], lhsT=L[:, 3 * HP:FW], rhs=B[:, N:FW], start=False, stop=True)


def _body(nc, tc, D, P, R, OUT):
    import contextlib

    ctx = contextlib.ExitStack()
    with ctx:
        consts = ctx.enter_context(tc.tile_pool(name="consts", bufs=1))
        stage = ctx.enter_context(tc.tile_pool(name="stage", bufs=5))
        pvec = ctx.enter_context(tc.tile_pool(name="pvec", bufs=34))
        work = ctx.enter_context(tc.tile_pool(name="work", bufs=6))
        scal = ctx.enter_context(tc.tile_pool(name="scal", bufs=8))
        blkp = ctx.enter_context(tc.tile_pool(name="blkp", bufs=2))
        ps_T = ctx.enter_context(tc.tile_pool(name="ps_T", bufs=3, space="PSUM"))
        ps_W = ctx.enter_context(tc.tile_pool(name="ps_W", bufs=3, space="PSUM"))
        ps_s = ctx.enter_context(tc.tile_pool(name="ps_s", bufs=1, space="PSUM"))

        # --- constants ---
        ones = consts.tile([HP, HP], F32)
        nc.vector.memset(ones, 1.0)
        sel = consts.tile([HP, 2 * BLK - 1], F32)  # windowed one-hot selector
        nc.vector.memset(sel, 0.0)
        nc.vector.memset(sel[:, BLK - 1:BLK], 1.0)
        id8 = consts.tile([BLK, BLK], F32)
        idt = consts.tile([BLK, BLK], mybir.dt.int32)
        nc.gpsimd.iota(idt, pattern=[[-1, BLK]], base=0, channel_multiplier=1)
        nc.vector.tensor_scalar(out=id8, in0=idt, scalar1=0, scalar2=None,
                                op0=ALU.is_equal)
        sgn4 = consts.tile([BLK, RANK], F32)  # -tau_k, tau = (-1)^k
        nc.vector.memset(sgn4[:, 0:1], -1.0)
        nc.vector.memset(sgn4[:, 1:2], 1.0)
        nc.vector.memset(sgn4[:, 2:3], -1.0)
        nc.vector.memset(sgn4[:, 3:4], 1.0)

        arena = ps_s.tile([HP, 512], F32, tag="arena", bufs=1)
        slot = [0]

        n_mols = D.shape[0]
        pending = None
        for b in range(n_mols // BLK):
            mols = list(range(b * BLK, (b + 1) * BLK))
            blk_state = []
            G = 4
            for g in range(0, BLK, G):
                grp = [_mol_head(nc, D, P, R, m, stage, pvec, work, scal)
                       for m in mols[g:g + G]]
                for k in range(RANK):
                    for st in grp:
                        _mol_rank(nc, st, k, work, scal, ps_T, ps_W, ones,
                                  arena, slot)
                blk_state.extend(grp)
            # tail of the PREVIOUS block: its serial solve chain overlaps
            # this block's mol pipelines instead of stalling them
            if pending is not None:
                _block_tail(nc, tc, OUT, pending[0], pending[1], work, scal,
                            blkp, ps_s, ones, sel, id8, sgn4)
            pending = (mols, blk_state)
        _block_tail(nc, tc, OUT, pending[0], pending[1], work, scal, blkp,
                    ps_s, ones, sel, id8, sgn4)


def _mol_head(nc, D, P, R, m, stage, pvec, work, scal):
    d_st = stage.tile([HP, FW], F32, tag="d_st")
    p_st = stage.tile([HP, FW], F32, tag="p_st")
    r_st = stage.tile([HP, FW], F32, tag="r_st")
    _load_e(nc, d_st, D, m)
    _load_e(nc, p_st, P, m)
    _load_e(nc, r_st, R, m)

    r_bf = work.tile([HP, FW], BF16, tag="r_bf", bufs=6)
    nc.scalar.copy(r_bf, r_st)

    partials = scal.tile([HP, NPART], F32, tag="partials", bufs=36)
    p_tiles = [pvec.tile([HP, FW], BF16, tag=f"p{k}", name=f"p{k}", bufs=34)
               for k in range(RANK)]

    nc.vector.tensor_sub(p_tiles[0], d_st, p_st)
    scr = work.tile([HP, FW], BF16, tag="scr", bufs=4)
    nc.scalar.activation(out=scr, in_=p_tiles[0], func=ACTF.Square,
                         accum_out=partials[:, 0:1])
    return {"partials": partials, "p": p_tiles, "r_bf": r_bf, "rn_prev": None}


def _mol_rank(nc, st, k, work, scal, ps_T, ps_W, ones, arena, slot):
    partials = st["partials"]
    p_tiles = st["p"]
    r_bf = st["r_bf"]
    pk = p_tiles[k]
    # T = p_k R   ->  W' = R T (E-layout PSUM)
    t_ps = ps_T.tile([HP, FW], F32, tag="t_ps")
    _sandwich(nc, t_ps, pk, r_bf)
    t_bf = work.tile([HP, FW], BF16, tag="t_bf", bufs=10)
    nc.scalar.copy(t_bf, t_ps)
    w_ps = ps_W.tile([HP, FW], F32, tag="w_ps")
    _sandwich(nc, w_ps, r_bf, t_bf)

    # S_k = <W', p_k>  (fused mult+reduce on DVE; junk out -> spent T bank)
    nc.vector.scalar_tensor_tensor(out=t_ps, in0=pk, scalar=1.0, in1=w_ps,
                                   op0=ALU.bypass, op1=ALU.mult,
                                   accum_out=partials[:, 2 * k + 1:2 * k + 2])

    if k == RANK - 1:
        # last rank: only <W',W'> is needed (O_33 identity), no p_4
        nc.scalar.activation(out=t_ps, in_=w_ps, func=ACTF.Square,
                             accum_out=partials[:, 8:9])
        return

    # broadcast [n_k, S_k] across partitions via ones-matmul into a
    # column-rotating single-bank arena (128 slots deep, no WAR stalls)
    c0 = 4 * (slot[0] % 128)
    slot[0] += 1
    bc = arena[:, c0:c0 + 4]
    nc.tensor.matmul(bc[:, 0:2], lhsT=ones,
                     rhs=partials[:, 2 * k:2 * k + 2], start=True, stop=True)
    rn = scal.tile([HP, 1], F32, tag="rn", bufs=12)
    nc.vector.reciprocal(rn, bc[:, 0:1])
    na = scal.tile([HP, 1], F32, tag="na", bufs=12)
    nc.scalar.activation(out=na, in_=bc[:, 1:2], func=ACTF.Copy, scale=rn)

    # sign-alternated recurrence: tiles hold (-1)^k p_k, so a'/b stay
    # positive and the axpy uses subtract; signs are undone in the tail
    if k == 0:
        nc.vector.scalar_tensor_tensor(out=p_tiles[1], in0=pk, scalar=na,
                                       in1=w_ps, op0=ALU.mult,
                                       op1=ALU.subtract)
    else:
        nb = scal.tile([HP, 1], F32, tag="nb", bufs=12)
        nc.scalar.activation(out=nb, in_=bc[:, 0:1], func=ACTF.Copy,
                             scale=st["rn_prev"])
        u1 = work.tile([HP, FW], BF16, tag="u1", bufs=10)
        nc.vector.scalar_tensor_tensor(out=u1, in0=pk, scalar=na,
                                       in1=w_ps, op0=ALU.mult,
                                       op1=ALU.subtract)
        t2 = work.tile([HP, FW], BF16, tag="t2", bufs=10)
        nc.scalar.activation(out=t2, in_=p_tiles[k - 1], func=ACTF.Copy,
                             scale=nb)
        nc.vector.tensor_sub(p_tiles[k + 1], u1, t2)
    st["rn_prev"] = rn

    # n_{k+1} = <p_{k+1}, p_{k+1}>  (junk out -> spent W bank);
    # alternate engines to balance ACT/DVE load
    if k % 2 == 0:
        nc.vector.scalar_tensor_tensor(out=w_ps, in0=p_tiles[k + 1], scalar=1.0,
                                       in1=p_tiles[k + 1], op0=ALU.bypass,
                                       op1=ALU.mult,
                                       accum_out=partials[:, 2 * k + 2:2 * k + 3])
    else:
        nc.scalar.activation(out=w_ps, in_=p_tiles[k + 1], func=ACTF.Square,
                             accum_out=partials[:, 2 * k + 2:2 * k + 3])


def _solve_sym4(nc, g, s):
    """Batched symmetric 4x4 solve on [BLK,1] column APs.

    g: [BLK, 14] tile, cols 0..9 = O (00,10,11,20,21,22,30,31,32,33),
    cols 10..13 = rhs c.  s: [BLK, 16] scratch.  Returns y col APs.
    """
    def col(t, i):
        return t[:, i:i + 1]

    a, bb, e, c, f, h, d, gg, i_, jj = (col(g, i) for i in range(10))
    r0, r1, r2, r3 = (col(g, 10 + i) for i in range(4))
    p0, p1, p2, p3 = (col(s, 4 + i) for i in range(4))
    l1, l2, l3 = (col(s, 8 + i) for i in range(3))
    m2, m3 = col(s, 16), col(s, 17)   # step-2 multipliers
    n3 = col(s, 18)                   # step-3 multiplier
    y0, y1, y2, y3 = (col(s, i) for i in range(4))

    mul = nc.vector.tensor_mul
    sub = nc.vector.tensor_sub
    rec = nc.vector.reciprocal

    # rotate scratch columns so independent row-updates of one pivot step
    # don't serialize on a shared temp (WAW); per-step multiplier columns
    # avoid the same hazard across pivot steps
    scr_cols = [11, 12, 13, 14, 15, 19, 20, 21, 22, 23]
    scr_i = [0]

    def upd(x, l, src):  # x -= l*src
        t0 = col(s, scr_cols[scr_i[0] % len(scr_cols)])
        scr_i[0] += 1
        mul(t0, l, src)
        sub(x, x, t0)

    rec(p0, a)
    mul(l1, bb, p0); mul(l2, c, p0); mul(l3, d, p0)
    upd(e, l1, bb); upd(f, l2, bb); upd(gg, l3, bb)
    upd(h, l2, c); upd(i_, l3, c); upd(jj, l3, d)
    upd(r1, l1, r0); upd(r2, l2, r0); upd(r3, l3, r0)

    rec(p1, e)
    mul(m2, f, p1); mul(m3, gg, p1)
    upd(h, m2, f); upd(i_, m3, f); upd(jj, m3, gg)
    upd(r2, m2, r1); upd(r3, m3, r1)

    rec(p2, h)
    mul(n3, i_, p2)
    upd(jj, n3, i_); upd(r3, n3, r2)

    rec(p3, jj)
    mul(y3, r3, p3)
    upd(r2, i_, y3); mul(y2, r2, p2)
    upd(r1, f, y2); upd(r1, gg, y3); mul(y1, r1, p1)
    upd(r0, bb, y1); upd(r0, c, y2); upd(r0, d, y3); mul(y0, r0, p0)
    return [y0, y1, y2, y3]


def _block_tail(nc, tc, OUT, mols, blk_state, work, scal, blkp, ps_s, ones,
                sel, id8, sgn4):
    # gather each mol's 9 partial sums into [BLK, 9] rows via selector matmuls
    gath = ps_s.tile([BLK, NPART], F32, tag="sm", bufs=1, name="gath")
    for j, st in enumerate(blk_state):
        nc.tensor.matmul(gath, lhsT=sel[:, BLK - 1 - j:2 * BLK - 1 - j],
                         rhs=st["partials"][:, 0:NPART],
                         start=(j == 0), stop=(j == len(blk_state) - 1))
    gb = blkp.tile([BLK, NPART], F32, tag="gb")
    nc.scalar.copy(gb, gath)

    # tridiagonal scalars -> O (10 cols) + c (4 cols)
    w = blkp.tile([BLK, 40], F32, tag="w")
    nv = gb[:, 0:8:2]                          # [8,4] n_0..n_3
    sv = gb[:, 1:8:2]                          # [8,4] S_0..S_3
    ww = gb[:, 8:9]                            # [8,1] <W'_3, W'_3>
    rn4 = w[:, 0:4]
    sq4 = w[:, 4:8]
    rsq4 = w[:, 8:12]
    av = w[:, 12:16]
    bv = w[:, 16:19]                           # beta_1..beta_3
    asq = w[:, 19:23]
    bz = w[:, 23:27]                           # [8,4]: 0, b1^2, b2^2, b3^2
    odg = w[:, 27:30]                          # O_00..O_22
    t3 = w[:, 30:33]
    o33 = w[:, 33:34]
    nc.vector.reciprocal(rn4, nv)
    nc.scalar.sqrt(sq4, nv)
    nc.vector.reciprocal(rsq4, sq4)
    nc.vector.tensor_mul(av, sv, rn4)                    # a' = S/n
    nc.vector.tensor_scalar(out=av, in0=av, scalar1=1.0, scalar2=None,
                            op0=ALU.subtract)            # a = a' - 1
    nc.vector.tensor_mul(bv, sq4[:, 1:4], rsq4[:, 0:3])  # beta_{k+1}
    nc.vector.tensor_mul(asq, av, av)
    nc.vector.memset(bz[:, 0:1], 0.0)
    nc.vector.tensor_mul(bz[:, 1:4], bv, bv)
    nc.vector.tensor_add(odg, asq[:, 0:3], bz[:, 0:3])
    nc.vector.tensor_add(odg, odg, bz[:, 1:4])           # O_kk, k=0..2
    # O_33 = (ww - 2 S_3 + n_3) / n_3
    nc.vector.tensor_add(o33, ww, nv[:, 3:4])
    nc.vector.tensor_scalar(out=w[:, 34:35], in0=sv[:, 3:4], scalar1=-2.0,
                            scalar2=None, op0=ALU.mult)
    nc.vector.tensor_add(o33, o33, w[:, 34:35])
    nc.vector.tensor_mul(o33, o33, rn4[:, 3:4])
    nc.vector.tensor_add(t3, av[:, 0:3], av[:, 1:4])
    nc.vector.tensor_mul(t3, t3, bv)                     # O_{k,k+1}

    g = blkp.tile([BLK, 14], F32, tag="g")
    # diag -> cols 0,2,5,9 ; off1 -> 1,4,8 ; off2 -> 3,7 ; O30 -> 6
    for i, cdst in enumerate((0, 2, 5)):
        nc.vector.tensor_copy(g[:, cdst:cdst + 1], odg[:, i:i + 1])
    nc.vector.tensor_copy(g[:, 9:10], o33)
    for i, cdst in enumerate((1, 4, 8)):
        nc.vector.tensor_copy(g[:, cdst:cdst + 1], t3[:, i:i + 1])
    nc.vector.tensor_mul(g[:, 3:4], bv[:, 0:1], bv[:, 1:2])
    nc.vector.tensor_mul(g[:, 7:8], bv[:, 1:2], bv[:, 2:3])
    nc.vector.memset(g[:, 6:7], 0.0)
    nc.vector.tensor_mul(g[:, 10:11], av[:, 0:1], sq4[:, 0:1])  # c0
    nc.vector.tensor_copy(g[:, 11:12], sq4[:, 1:2])             # c1
    nc.vector.memset(g[:, 12:14], 0.0)

    s_sb = blkp.tile([BLK, 24], F32, tag="s_sb")
    ys = _solve_sym4(nc, g, s_sb)
    yneg = blkp.tile([BLK, RANK], F32, tag="yneg")
    for i in range(RANK):
        nc.vector.tensor_copy(yneg[:, i:i + 1], ys[i])
    nc.vector.tensor_mul(yneg, yneg, rsq4)
    nc.vector.tensor_mul(yneg, yneg, sgn4)

    ymask = blkp.tile([BLK, BLK * RANK], F32, tag="ymask")
    for j in range(BLK):
        nc.scalar.activation(out=ymask[:, RANK * j:RANK * (j + 1)], in_=yneg,
                             func=ACTF.Copy, scale=id8[:, j:j + 1])
    ybc = ps_s.tile([HP, BLK * RANK], F32, tag="sm", bufs=1, name="ybc")
    nc.tensor.matmul(ybc, lhsT=ones[0:BLK, :], rhs=ymask, start=True, stop=True)
    yb = blkp.tile([HP, BLK * RANK], F32, tag="yb")
    nc.scalar.copy(yb, ybc)

    for j, (m, st) in enumerate(zip(mols, blk_state)):
        p = st["p"]
        acc = work.tile([HP, FW], F32, tag="acc")
        nc.scalar.activation(out=acc, in_=p[0], func=ACTF.Copy,
                             scale=yb[:, 4 * j:4 * j + 1])
        acc1 = work.tile([HP, FW], F32, tag="acc1")
        nc.vector.scalar_tensor_tensor(out=acc1, in0=p[1], scalar=yb[:, 4 * j + 1:4 * j + 2],
                                       in1=acc, op0=ALU.mult, op1=ALU.add)
        t2c = work.tile([HP, FW], F32, tag="t2c")
        nc.vector.tensor_scalar(out=t2c, in0=p[2],
                                scalar1=yb[:, 4 * j + 2:4 * j + 3],
                                scalar2=None, op0=ALU.mult)
        acc2 = work.tile([HP, FW], F32, tag="acc2")
        nc.vector.tensor_add(acc2, acc1, t2c)
        acc3 = work.tile([HP, FW], F32, tag="acc3")
        nc.vector.scalar_tensor_tensor(out=acc3, in0=p[3],
                                       scalar=yb[:, 4 * j + 3:4 * j + 4],
                                       in1=acc2, op0=ALU.mult, op1=ALU.add)
        _store_e(nc, OUT, m, acc3)


_NC_CACHE = None


def _get_nc():
    global _NC_CACHE
    if _NC_CACHE is None:
        _NC_CACHE = build_core_kernel()
    return _NC_CACHE


def kernel(D, P, R, max_rank=4, _trace=False):
    D = np.ascontiguousarray(D, dtype=np.float32)
    P = np.ascontiguousarray(P, dtype=np.float32)
    R = np.ascontiguousarray(R, dtype=np.float32)
    nc = _get_nc()
    in_maps = []
    for i in range(NCORES):
        sl = slice(i * MPC, (i + 1) * MPC)
        in_maps.append({"D": D[sl], "P": P[sl], "Rm": R[sl]})
    res = run_bass_kernel_spmd(nc, in_maps, core_ids=list(range(NCORES)),
                               trace=_trace)
    out = np.concatenate([r["OUT"] for r in res.results], axis=0)
    if _trace:
        kernel.last_exec_time_ns = res.exec_time_ns
        kernel.last_trace = res.instructions_and_trace
    return out


# revision 40
# speedup vs baseline: 1.0064x; 1.0064x over previous
"""XL-BOMD rank-4 Krylov propagation (EnergyXL) on 8 TRN2 NeuronCores.

Data-parallel over molecules: 512 mols -> 64 per core.  The operator
A(v) = R v R - v is self-adjoint w.r.t. the Frobenius inner product, so
the reference's full Gram-Schmidt chain collapses to an (unnormalized)
Lanczos 3-term recurrence:

  p_0 = D - P,  n_k = <p_k,p_k>
  W'  = R p_k R              (bf16 PE sandwiches, fp32 PSUM accum)
  a'_k = <W', p_k> / n_k,  b_k = n_k / n_{k-1}
  p_{k+1} = W' - a'_k p_k - b_k p_{k-1}

The final dP2dt2 = -V (W^T W)^-1 W^T dDS is basis-invariant over the
Krylov subspace; O/c are reconstructed from the tridiagonal scalars
(a_k = a'_k - 1, beta_k = sqrt(n_k/n_{k-1})); rank 3 needs no p_4:
O_33 = (<W',W'> - 2 S_3 + n_3)/n_3.  Batched symmetric 4x4 solve per
8-mol block, then out = -sum_k y_k/sqrt(n_k) p_k.

Layout: each 192x192 matrix lives as one [96, 384] SBUF tile (rows
0:96 -> cols 0:192, rows 96:192 -> cols 192:384), so every elementwise
op / reduction is a single instruction.  Matmuls split into 4 [96,96]
weight chunks x 192-wide moving ops into a single [96,384] PSUM bank.
Vectors are stored bf16 (validated 2.6e-3 rel err vs 2e-2 budget);
all reductions accumulate fp32.
"""

import sys

sys.path.insert(0, "/opt/trn_rl_repo")

import numpy as np

import concourse.bass as bass
import concourse.bacc as bacc
import concourse.tile as tile
from concourse import mybir
from concourse.bass_utils import run_bass_kernel_spmd

F32 = mybir.dt.float32
BF16 = mybir.dt.bfloat16
ALU = mybir.AluOpType
ACTF = mybir.ActivationFunctionType

NMOL, N, RANK = 512, 192, 4
NCORES = 8
MPC = NMOL // NCORES  # 64 molecules per core
HP = 96               # partitions per tile (192 rows in 2 col groups)
FW = 384              # free width: 2 x 192
BLK = 16              # molecules per solve block

# partials cols (per mol, [96, 9]): n_k at 2k (k=0..3), S_k at 2k+1, WW at 8
NPART = 9


def build_core_kernel(n_mols=MPC):
    nc = bacc.Bacc(None, target_bir_lowering=False, enable_partition_id=False)
    D = nc.dram_tensor("D", [n_mols, N, N], F32, kind="ExternalInput")
    P = nc.dram_tensor("P", [n_mols, N, N], F32, kind="ExternalInput")
    R = nc.dram_tensor("Rm", [n_mols, N, N], F32, kind="ExternalInput")
    OUT = nc.dram_tensor("OUT", [n_mols, N, N], F32, kind="ExternalOutput")

    with tile.TileContext(nc) as tc:
        _body(nc, tc, D, P, R, OUT)
    nc.finalize()
    return nc


def _load_e(nc, t, X, m):
    """DMA X[m] (192x192 DRAM) into E-layout tile t [96, 384]."""
    nc.sync.dma_start(out=t[:, 0:N], in_=X[m, 0:HP, :])
    nc.sync.dma_start(out=t[:, N:FW], in_=X[m, HP:N, :])


def _store_e(nc, X, m, t):
    nc.sync.dma_start(out=X[m, 0:HP, :], in_=t[:, 0:N])
    nc.sync.dma_start(out=X[m, HP:N, :], in_=t[:, N:FW])


def _sandwich(nc, ps, L, B):
    """ps[96,384] (PSUM) = (L @ B) in E-layout; L symmetric, both bf16.

    out rows 0:96 -> ps[:,0:192], rows 96:192 -> ps[:,192:384];
    contraction split over row groups 0:96 / 96:192.
    """
    mm = nc.tensor.matmul
    mm(ps[:, 0:N], lhsT=L[:, 0:HP], rhs=B[:, 0:N], start=True, stop=False)
    mm(ps[:, 0:N], lhsT=L[:, 2 * HP:3 * HP], rhs=B[:, N:FW], start=False, stop=True)
    mm(ps[:, N:FW], lhsT=L[:, HP:2 * HP], rhs=B[:, 0:N], start=True, stop=False)
    mm(ps[:, N:FW], lhsT=L[:, 3 * HP:FW], rhs=B[:, N:FW], start=False, stop=True)


def _body(nc, tc, D, P, R, OUT):
    import contextlib

    ctx = contextlib.ExitStack()
    with ctx:
        consts = ctx.enter_context(tc.tile_pool(name="consts", bufs=1))
        stage = ctx.enter_context(tc.tile_pool(name="stage", bufs=4))
        pvec = ctx.enter_context(tc.tile_pool(name="pvec", bufs=34))
        work = ctx.enter_context(tc.tile_pool(name="work", bufs=6))
        scal = ctx.enter_context(tc.tile_pool(name="scal", bufs=8))
        blkp = ctx.enter_context(tc.tile_pool(name="blkp", bufs=2))
        ps_T = ctx.enter_context(tc.tile_pool(name="ps_T", bufs=3, space="PSUM"))
        ps_W = ctx.enter_context(tc.tile_pool(name="ps_W", bufs=3, space="PSUM"))
        ps_s = ctx.enter_context(tc.tile_pool(name="ps_s", bufs=1, space="PSUM"))

        # --- constants ---
        ones = consts.tile([HP, HP], F32)
        nc.vector.memset(ones, 1.0)
        sel = consts.tile([HP, 2 * BLK - 1], F32)  # windowed one-hot selector
        nc.vector.memset(sel, 0.0)
        nc.vector.memset(sel[:, BLK - 1:BLK], 1.0)
        id8 = consts.tile([BLK, BLK], F32)
        idt = consts.tile([BLK, BLK], mybir.dt.int32)
        nc.gpsimd.iota(idt, pattern=[[-1, BLK]], base=0, channel_multiplier=1)
        nc.vector.tensor_scalar(out=id8, in0=idt, scalar1=0, scalar2=None,
                                op0=ALU.is_equal)
        sgn4 = consts.tile([BLK, RANK], F32)  # -tau_k, tau = (-1)^k
        nc.vector.memset(sgn4[:, 0:1], -1.0)
        nc.vector.memset(sgn4[:, 1:2], 1.0)
        nc.vector.memset(sgn4[:, 2:3], -1.0)
        nc.vector.memset(sgn4[:, 3:4], 1.0)

        arena = ps_s.tile([HP, 512], F32, tag="arena", bufs=1)
        slot = [0]

        n_mols = D.shape[0]
        pending = None
        for b in range(n_mols // BLK):
            mols = list(range(b * BLK, (b + 1) * BLK))
            blk_state = []
            G = 4
            for g in range(0, BLK, G):
                grp = [_mol_head(nc, D, P, R, m, stage, pvec, work, scal)
                       for m in mols[g:g + G]]
                for k in range(RANK):
                    for st in grp:
                        _mol_rank(nc, st, k, work, scal, ps_T, ps_W, ones,
                                  arena, slot)
                blk_state.extend(grp)
            # tail of the PREVIOUS block: its serial solve chain overlaps
            # this block's mol pipelines instead of stalling them
            if pending is not None:
                _block_tail(nc, tc, OUT, pending[0], pending[1], work, scal,
                            blkp, ps_s, ones, sel, id8, sgn4)
            pending = (mols, blk_state)
        _block_tail(nc, tc, OUT, pending[0], pending[1], work, scal, blkp,
                    ps_s, ones, sel, id8, sgn4)


def _mol_head(nc, D, P, R, m, stage, pvec, work, scal):
    d_st = stage.tile([HP, FW], F32, tag="d_st")
    p_st = stage.tile([HP, FW], F32, tag="p_st")
    r_st = stage.tile([HP, FW], F32, tag="r_st")
    _load_e(nc, d_st, D, m)
    _load_e(nc, p_st, P, m)
    _load_e(nc, r_st, R, m)

    r_bf = work.tile([HP, FW], BF16, tag="r_bf", bufs=6)
    nc.scalar.copy(r_bf, r_st)

    partials = scal.tile([HP, NPART], F32, tag="partials", bufs=36)
    p_tiles = [pvec.tile([HP, FW], BF16, tag=f"p{k}", name=f"p{k}", bufs=34)
               for k in range(RANK)]

    nc.vector.tensor_sub(p_tiles[0], d_st, p_st)
    scr = work.tile([HP, FW], BF16, tag="scr", bufs=4)
    nc.scalar.activation(out=scr, in_=p_tiles[0], func=ACTF.Square,
                         accum_out=partials[:, 0:1])
    return {"partials": partials, "p": p_tiles, "r_bf": r_bf, "rn_prev": None}


def _mol_rank(nc, st, k, work, scal, ps_T, ps_W, ones, arena, slot):
    partials = st["partials"]
    p_tiles = st["p"]
    r_bf = st["r_bf"]
    pk = p_tiles[k]
    # T = p_k R   ->  W' = R T (E-layout PSUM)
    t_ps = ps_T.tile([HP, FW], F32, tag="t_ps")
    _sandwich(nc, t_ps, pk, r_bf)
    t_bf = work.tile([HP, FW], BF16, tag="t_bf", bufs=10)
    nc.scalar.copy(t_bf, t_ps)
    w_ps = ps_W.tile([HP, FW], F32, tag="w_ps")
    _sandwich(nc, w_ps, r_bf, t_bf)

    # S_k = <W', p_k>  (fused mult+reduce on DVE; junk out -> spent T bank)
    nc.vector.scalar_tensor_tensor(out=t_ps, in0=pk, scalar=1.0, in1=w_ps,
                                   op0=ALU.bypass, op1=ALU.mult,
                                   accum_out=partials[:, 2 * k + 1:2 * k + 2])

    if k == RANK - 1:
        # last rank: only <W',W'> is needed (O_33 identity), no p_4
        nc.scalar.activation(out=t_ps, in_=w_ps, func=ACTF.Square,
                             accum_out=partials[:, 8:9])
        return

    # broadcast [n_k, S_k] across partitions via ones-matmul into a
    # column-rotating single-bank arena (128 slots deep, no WAR stalls)
    c0 = 4 * (slot[0] % 128)
    slot[0] += 1
    bc = arena[:, c0:c0 + 4]
    nc.tensor.matmul(bc[:, 0:2], lhsT=ones,
                     rhs=partials[:, 2 * k:2 * k + 2], start=True, stop=True)
    rn = scal.tile([HP, 1], F32, tag="rn", bufs=12)
    nc.vector.reciprocal(rn, bc[:, 0:1])
    na = scal.tile([HP, 1], F32, tag="na", bufs=12)
    nc.scalar.activation(out=na, in_=bc[:, 1:2], func=ACTF.Copy, scale=rn)

    # sign-alternated recurrence: tiles hold (-1)^k p_k, so a'/b stay
    # positive and the axpy uses subtract; signs are undone in the tail
    if k == 0:
        nc.vector.scalar_tensor_tensor(out=p_tiles[1], in0=pk, scalar=na,
                                       in1=w_ps, op0=ALU.mult,
                                       op1=ALU.subtract)
    else:
        nb = scal.tile([HP, 1], F32, tag="nb", bufs=12)
        nc.scalar.activation(out=nb, in_=bc[:, 0:1], func=ACTF.Copy,
                             scale=st["rn_prev"])
        u1 = work.tile([HP, FW], BF16, tag="u1", bufs=10)
        nc.vector.scalar_tensor_tensor(out=u1, in0=pk, scalar=na,
                                       in1=w_ps, op0=ALU.mult,
                                       op1=ALU.subtract)
        t2 = work.tile([HP, FW], BF16, tag="t2", bufs=10)
        nc.scalar.activation(out=t2, in_=p_tiles[k - 1], func=ACTF.Copy,
                             scale=nb)
        nc.vector.tensor_sub(p_tiles[k + 1], u1, t2)
    st["rn_prev"] = rn

    # n_{k+1} = <p_{k+1}, p_{k+1}>  (junk out -> spent W bank);
    # alternate engines to balance ACT/DVE load
    if k % 2 == 0:
        nc.vector.scalar_tensor_tensor(out=w_ps, in0=p_tiles[k + 1], scalar=1.0,
                                       in1=p_tiles[k + 1], op0=ALU.bypass,
                                       op1=ALU.mult,
                                       accum_out=partials[:, 2 * k + 2:2 * k + 3])
    else:
        nc.scalar.activation(out=w_ps, in_=p_tiles[k + 1], func=ACTF.Square,
                             accum_out=partials[:, 2 * k + 2:2 * k + 3])


def _solve_sym4(nc, g, s):
    """Batched symmetric 4x4 solve on [BLK,1] column APs.

    g: [BLK, 14] tile, cols 0..9 = O (00,10,11,20,21,22,30,31,32,33),
    cols 10..13 = rhs c.  s: [BLK, 16] scratch.  Returns y col APs.
    """
    def col(t, i):
        return t[:, i:i + 1]

    a, bb, e, c, f, h, d, gg, i_, jj = (col(g, i) for i in range(10))
    r0, r1, r2, r3 = (col(g, 10 + i) for i in range(4))
    p0, p1, p2, p3 = (col(s, 4 + i) for i in range(4))
    l1, l2, l3 = (col(s, 8 + i) for i in range(3))
    m2, m3 = col(s, 16), col(s, 17)   # step-2 multipliers
    n3 = col(s, 18)                   # step-3 multiplier
    y0, y1, y2, y3 = (col(s, i) for i in range(4))

    mul = nc.vector.tensor_mul
    sub = nc.vector.tensor_sub
    rec = nc.vector.reciprocal

    # rotate scratch columns so independent row-updates of one pivot step
    # don't serialize on a shared temp (WAW); per-step multiplier columns
    # avoid the same hazard across pivot steps
    scr_cols = [11, 12, 13, 14, 15, 19, 20, 21, 22, 23]
    scr_i = [0]

    def upd(x, l, src):  # x -= l*src
        t0 = col(s, scr_cols[scr_i[0] % len(scr_cols)])
        scr_i[0] += 1
        mul(t0, l, src)
        sub(x, x, t0)

    rec(p0, a)
    mul(l1, bb, p0); mul(l2, c, p0); mul(l3, d, p0)
    upd(e, l1, bb); upd(f, l2, bb); upd(gg, l3, bb)
    upd(h, l2, c); upd(i_, l3, c); upd(jj, l3, d)
    upd(r1, l1, r0); upd(r2, l2, r0); upd(r3, l3, r0)

    rec(p1, e)
    mul(m2, f, p1); mul(m3, gg, p1)
    upd(h, m2, f); upd(i_, m3, f); upd(jj, m3, gg)
    upd(r2, m2, r1); upd(r3, m3, r1)

    rec(p2, h)
    mul(n3, i_, p2)
    upd(jj, n3, i_); upd(r3, n3, r2)

    rec(p3, jj)
    mul(y3, r3, p3)
    upd(r2, i_, y3); mul(y2, r2, p2)
    upd(r1, f, y2); upd(r1, gg, y3); mul(y1, r1, p1)
    upd(r0, bb, y1); upd(r0, c, y2); upd(r0, d, y3); mul(y0, r0, p0)
    return [y0, y1, y2, y3]


def _block_tail(nc, tc, OUT, mols, blk_state, work, scal, blkp, ps_s, ones,
                sel, id8, sgn4):
    # gather each mol's 9 partial sums into [BLK, 9] rows via selector matmuls
    gath = ps_s.tile([BLK, NPART], F32, tag="sm", bufs=1, name="gath")
    for j, st in enumerate(blk_state):
        nc.tensor.matmul(gath, lhsT=sel[:, BLK - 1 - j:2 * BLK - 1 - j],
                         rhs=st["partials"][:, 0:NPART],
                         start=(j == 0), stop=(j == len(blk_state) - 1))
    gb = blkp.tile([BLK, NPART], F32, tag="gb")
    nc.scalar.copy(gb, gath)

    # tridiagonal scalars -> O (10 cols) + c (4 cols)
    w = blkp.tile([BLK, 40], F32, tag="w")
    nv = gb[:, 0:8:2]                          # [8,4] n_0..n_3
    sv = gb[:, 1:8:2]                          # [8,4] S_0..S_3
    ww = gb[:, 8:9]                            # [8,1] <W'_3, W'_3>
    rn4 = w[:, 0:4]
    sq4 = w[:, 4:8]
    rsq4 = w[:, 8:12]
    av = w[:, 12:16]
    bv = w[:, 16:19]                           # beta_1..beta_3
    asq = w[:, 19:23]
    bz = w[:, 23:27]                           # [8,4]: 0, b1^2, b2^2, b3^2
    odg = w[:, 27:30]                          # O_00..O_22
    t3 = w[:, 30:33]
    o33 = w[:, 33:34]
    nc.vector.reciprocal(rn4, nv)
    nc.scalar.sqrt(sq4, nv)
    nc.vector.reciprocal(rsq4, sq4)
    nc.vector.tensor_mul(av, sv, rn4)                    # a' = S/n
    nc.vector.tensor_scalar(out=av, in0=av, scalar1=1.0, scalar2=None,
                            op0=ALU.subtract)            # a = a' - 1
    nc.vector.tensor_mul(bv, sq4[:, 1:4], rsq4[:, 0:3])  # beta_{k+1}
    nc.vector.tensor_mul(asq, av, av)
    nc.vector.memset(bz[:, 0:1], 0.0)
    nc.vector.tensor_mul(bz[:, 1:4], bv, bv)
    nc.vector.tensor_add(odg, asq[:, 0:3], bz[:, 0:3])
    nc.vector.tensor_add(odg, odg, bz[:, 1:4])           # O_kk, k=0..2
    # O_33 = (ww - 2 S_3 + n_3) / n_3
    nc.vector.tensor_add(o33, ww, nv[:, 3:4])
    nc.vector.tensor_scalar(out=w[:, 34:35], in0=sv[:, 3:4], scalar1=-2.0,
                            scalar2=None, op0=ALU.mult)
    nc.vector.tensor_add(o33, o33, w[:, 34:35])
    nc.vector.tensor_mul(o33, o33, rn4[:, 3:4])
    nc.vector.tensor_add(t3, av[:, 0:3], av[:, 1:4])
    nc.vector.tensor_mul(t3, t3, bv)                     # O_{k,k+1}

    g = blkp.tile([BLK, 14], F32, tag="g")
    # diag -> cols 0,2,5,9 ; off1 -> 1,4,8 ; off2 -> 3,7 ; O30 -> 6
    for i, cdst in enumerate((0, 2, 5)):
        nc.vector.tensor_copy(g[:, cdst:cdst + 1], odg[:, i:i + 1])
    nc.vector.tensor_copy(g[:, 9:10], o33)
    for i, cdst in enumerate((1, 4, 8)):
        nc.vector.tensor_copy(g[:, cdst:cdst + 1], t3[:, i:i + 1])
    nc.vector.tensor_mul(g[:, 3:4], bv[:, 0:1], bv[:, 1:2])
    nc.vector.tensor_mul(g[:, 7:8], bv[:, 1:2], bv[:, 2:3])
    nc.vector.memset(g[:, 6:7], 0.0)
    nc.vector.tensor_mul(g[:, 10:11], av[:, 0:1], sq4[:, 0:1])  # c0
    nc.vector.tensor_copy(g[:, 11:12], sq4[:, 1:2])             # c1
    nc.vector.memset(g[:, 12:14], 0.0)

    s_sb = blkp.tile([BLK, 24], F32, tag="s_sb")
    ys = _solve_sym4(nc, g, s_sb)
    yneg = blkp.tile([BLK, RANK], F32, tag="yneg")
    for i in range(RANK):
        nc.vector.tensor_copy(yneg[:, i:i + 1], ys[i])
    nc.vector.tensor_mul(yneg, yneg, rsq4)
    nc.vector.tensor_mul(yneg, yneg, sgn4)

    ymask = blkp.tile([BLK, BLK * RANK], F32, tag="ymask")
    for j in range(BLK):
        nc.scalar.activation(out=ymask[:, RANK * j:RANK * (j + 1)], in_=yneg,
                             func=ACTF.Copy, scale=id8[:, j:j + 1])
    ybc = ps_s.tile([HP, BLK * RANK], F32, tag="sm", bufs=1, name="ybc")
    nc.tensor.matmul(ybc, lhsT=ones[0:BLK, :], rhs=ymask, start=True, stop=True)
    yb = blkp.tile([HP, BLK * RANK], F32, tag="yb")
    nc.scalar.copy(yb, ybc)

    for j, (m, st) in enumerate(zip(mols, blk_state)):
        p = st["p"]
        acc = work.tile([HP, FW], F32, tag="acc")
        nc.scalar.activation(out=acc, in_=p[0], func=ACTF.Copy,
                             scale=yb[:, 4 * j:4 * j + 1])
        acc1 = work.tile([HP, FW], F32, tag="acc1")
        nc.vector.scalar_tensor_tensor(out=acc1, in0=p[1], scalar=yb[:, 4 * j + 1:4 * j + 2],
                                       in1=acc, op0=ALU.mult, op1=ALU.add)
        t2c = work.tile([HP, FW], F32, tag="t2c")
        nc.vector.tensor_scalar(out=t2c, in0=p[2],
                                scalar1=yb[:, 4 * j + 2:4 * j + 3],
                                scalar2=None, op0=ALU.mult)
        acc2 = work.tile([HP, FW], F32, tag="acc2")
        nc.vector.tensor_add(acc2, acc1, t2c)
        acc3 = work.tile([HP, FW], F32, tag="acc3")
        nc.vector.scalar_tensor_tensor(out=acc3, in0=p[3],
                                       scalar=yb[:, 4 * j + 3:4 * j + 4],
                                       in1=acc2, op0=ALU.mult, op1=ALU.add)
        _store_e(nc, OUT, m, acc3)


_NC_CACHE = None


def _get_nc():
    global _NC_CACHE
    if _NC_CACHE is None:
        _NC_CACHE = build_core_kernel()
    return _NC_CACHE


def kernel(D, P, R, max_rank=4, _trace=False):
    D = np.ascontiguousarray(D, dtype=np.float32)
    P = np.ascontiguousarray(P, dtype=np.float32)
    R = np.ascontiguousarray(R, dtype=np.float32)
    nc = _get_nc()
    in_maps = []
    for i in range(NCORES):
        sl = slice(i * MPC, (i + 1) * MPC)
        in_maps.append({"D": D[sl], "P": P[sl], "Rm": R[sl]})
    res = run_bass_kernel_spmd(nc, in_maps, core_ids=list(range(NCORES)),
                               trace=_trace)
    out = np.concatenate([r["OUT"] for r in res.results], axis=0)
    if _trace:
        kernel.last_exec_time_ns = res.exec_time_ns
        kernel.last_trace = res.instructions_and_trace
    return out
